# revision 2
# baseline (speedup 1.0000x reference)
"""HSTU block kernel for Trainium2, 8-core data-parallel over batch.

All matmul operands are f16 (4x PE throughput vs fp32; fp32 accumulation in
PSUM). Layouts avoid on-device transposes:
  - x ships as xT [D, N] f16 (stats + proj rhs) and row-major f16 (residual,
    with b_o and the output pad mask folded in on host).
  - proj is produced transposed (projT [E, N] f16) for u/q/k; v row-major f16.
  - qk logits in [key m, query n] layout; the rel-bias (pos table + ts_w
    bucket lookup, both index-derived tables expanded on host, causal mask
    baked in as -100: silu(-100) == 0 in f16) is preloaded into PSUM via an
    f16 identity matmul and the qk matmul accumulates on top.
  - attention runs as two query-column sweeps (cols [0,512) then [512,1024)),
    head-pair major with key tiles innermost and 3-deep PSUM window
    pipelining. attn@v matmuls skip below-diagonal columns. Each column half
    finishes its LN-a stats, u-product, output projection and store tucked
    behind the other half's sweep.
"""

import sys

sys.path.insert(0, "/opt/trn_rl_repo")

import numpy as np

import concourse.bass as bass
import concourse.tile as tile
import concourse.mybir as mybir
from concourse import bacc
from concourse.masks import make_identity

B, N, D = 8, 1024, 512
H, DV, DQ = 8, 64, 64
E = 2 * H * DV + 2 * H * DQ  # 2048
EPS = 1e-5
P = 128
NT = N // P  # 8 row tiles
F32 = mybir.dt.float32
F16 = mybir.dt.float16

_cache = {}


def _bucket(d):
    d = np.maximum(np.abs(d), 1).astype(np.float32)
    return np.clip((np.log(d) / np.float32(0.301)).astype(np.int32), 0, 128)


def _build(fold_ga, has_bb, haspad):
    nc = bacc.Bacc()
    d = {}
    for name, shape, dt in [
        ("xT16", [D, N], F16), ("xr", [N, D], F16),
        ("w_uqk", [P, 12 * 4 * P], F16),
        ("w_uvv", [P, 4 * 512], F16), ("wo_w", [P, 4 * 512], F16),
        ("bU_col", [P, E // P], F32), ("bUv16", [P, DV * H], F16),
        ("ga_col", [P, 4], F32), ("bb_col", [P, 4], F32),
        ("vscale_col", [P, NT], F32), ("padout_col", [P, NT], F32),
        ("posacc16", [P, 4608], F16),
    ]:
        d[name] = nc.dram_tensor(name, shape, dt, kind="ExternalInput")
    out_t = nc.dram_tensor("out", [N, D], F32, kind="ExternalOutput")

    widths = [N - P * r for r in range(NT)]
    offs = np.concatenate([[0], np.cumsum(widths)]).astype(int)
    uqk_tiles = [0, 1, 2, 3] + list(range(8, 16))

    from contextlib import ExitStack
    with tile.TileContext(nc) as tc, ExitStack() as ctx:
        io = ctx.enter_context(tc.tile_pool(name="io", bufs=1))
        pools = ctx.enter_context(tc.tile_pool(name="work", bufs=4))
        kpool = ctx.enter_context(tc.tile_pool(name="kpool", bufs=4))
        qpool = ctx.enter_context(tc.tile_pool(name="qpool", bufs=4))
        psum = ctx.enter_context(tc.tile_pool(name="psum", bufs=2, space="PSUM"))
        psqk = ctx.enter_context(tc.tile_pool(name="psqk", bufs=3, space="PSUM"))

        # ---- persistent SBUF tensors ----
        xT = [io.tile([P, N], F16, tag=f"xT{s}", name=f"xT{s}") for s in range(4)]
        for s in range(4):
            nc.sync.dma_start(xT[s][:], d["xT16"][P * s:P * s + P, :])
        w_uqk = io.tile([P, 12 * 4 * P], F16, tag="w_uqk")
        nc.sync.dma_start(w_uqk[:], d["w_uqk"][:])
        w_uvv = io.tile([P, 4 * 512], F16, tag="w_uvv")
        nc.sync.dma_start(w_uvv[:], d["w_uvv"][:])
        wo = io.tile([P, 4 * 512], F16, tag="wo")
        nc.sync.dma_start(wo[:], d["wo_w"][:])
        small = {}
        for nm, sh in [("bU_col", [P, E // P]),
                       ("ga_col", [P, 4]), ("bb_col", [P, 4]),
                       ("vscale_col", [P, NT]), ("padout_col", [P, NT])]:
            small[nm] = io.tile(sh, F32, tag=nm, name=nm)
            nc.sync.dma_start(small[nm][:], d[nm][:])
        bUv16 = io.tile([P, DV * H], F16, tag="bUv16")
        nc.sync.dma_start(bUv16[:], d["bUv16"][:])
        acc = [io.tile([P, widths[r]], F16, tag=f"acc{r}", name=f"acc{r}")
               for r in range(NT)]
        for r in range(NT):
            nc.sync.dma_start(acc[r][:], d["posacc16"][:, offs[r]:offs[r + 1]])
        xtile = [io.tile([P, D], F16, tag=f"xt{t}", name=f"xt{t}") for t in range(NT)]
        for t in range(NT):
            nc.sync.dma_start(xtile[t][:], d["xr"][P * t:P * t + P, :])

        ident = io.tile([P, P], F16, tag="ident")
        make_identity(nc, ident[:])
        ones_col = io.tile([P, 1], F16, tag="ones_col")
        nc.vector.memset(ones_col[:], 1.0)
        ones_row = io.tile([1, P], F16, tag="ones_row")
        nc.vector.memset(ones_row[:], 1.0)
        eps_t = io.tile([1, 1], F32, tag="eps_t")
        nc.vector.memset(eps_t[:], EPS)

        # ---- layernorm stats of x: four [1,512] accumulators packed into
        # one 2-bank PSUM tile (partitions 0/32 x column halves) ----
        stats_x = psqk.tile([P, 1024], F32, tag="qk", name="stats_x")
        s1p = [stats_x[32 * c:32 * c + 1, 0:512] for c in range(2)]
        s2p = [stats_x[32 * c:32 * c + 1, 512:1024] for c in range(2)]
        for s in range(4):
            sq = kpool.tile([P, N], F16, tag="kt", name="sq")
            nc.scalar.activation(sq[:], xT[s][:],
                                 mybir.ActivationFunctionType.Square)
            for c in range(2):
                nc.tensor.matmul(s1p[c], ones_col[:],
                                 xT[s][:, 512 * c:512 * c + 512],
                                 start=(s == 0), stop=(s == 3),
                                 skip_group_check=True)
                nc.tensor.matmul(s2p[c], ones_col[:],
                                 sq[:, 512 * c:512 * c + 512],
                                 start=(s == 0), stop=(s == 3),
                                 skip_group_check=True)

        def ln_half(s1c, s2c, c, tagpfx):
            """mu16, rs16 [1, 512] f16 for column half c from psum sums."""
            mu16 = io.tile([1, 512], F16, tag=f"{tagpfx}mu16_{c}")
            var = pools.tile([1, 512], F32, tag="v32", name="var")
            nc.vector.tensor_scalar_mul(mu16[:], s1c, 1.0 / D)
            mu2 = pools.tile([1, 512], F32, tag="v32", name="mu2")
            nc.vector.tensor_tensor(mu2[:], mu16[:], mu16[:], mybir.AluOpType.mult)
            nc.vector.scalar_tensor_tensor(var[:], s2c, 1.0 / D, mu2[:],
                                           mybir.AluOpType.mult,
                                           mybir.AluOpType.subtract)
            nc.scalar.activation(var[:], var[:],
                                 mybir.ActivationFunctionType.Sqrt,
                                 bias=eps_t[:], scale=1.0)
            rs16 = io.tile([1, 512], F16, tag=f"{tagpfx}rs16_{c}")
            with nc.allow_low_precision(reason="rs ~ O(1); f16 rel err ok"):
                nc.vector.reciprocal(rs16[:], var[:])
            return mu16, rs16

        mur = io.tile([P, N], F16, tag="mur")
        rsr = io.tile([P, N], F16, tag="rsr")
        xnt = xT  # normalized in place, per column half
        projT = {}
        for t in uqk_tiles:
            projT[t] = io.tile([P, N], F16, tag=f"pT{t}", name=f"pT{t}")
        vt = [io.tile([P, D], F16, tag=f"v{r}", name=f"v{r}") for r in range(NT)]
        vraw = [None] * NT

        def ln_xnt_half(c):
            mu16, rs16 = ln_half(s1p[c], s2p[c], c, "x")
            nc.gpsimd.partition_broadcast(mur[:, 512 * c:512 * c + 512], mu16[:])
            nc.gpsimd.partition_broadcast(rsr[:, 512 * c:512 * c + 512], rs16[:])
            cs = slice(512 * c, 512 * c + 512)
            for s in range(4):
                nc.vector.tensor_tensor(xnt[s][:, cs], xT[s][:, cs],
                                        mur[:, cs], mybir.AluOpType.subtract)
                nc.vector.tensor_tensor(xnt[s][:, cs], xnt[s][:, cs],
                                        rsr[:, cs], mybir.AluOpType.mult)

        def proj_half(c):
            """u/q/k projection cols [512c, 512c+512) and v tiles 4c..4c+3.
            (vscale multiplies are emitted separately, off the DVE bias path)"""
            cs = slice(512 * c, 512 * c + 512)
            for ti, t in enumerate(uqk_tiles):
                pt = psum.tile([P, 512], F32, tag="proj", name="pt")
                for s in range(4):
                    nc.tensor.matmul(pt[:], w_uqk[:, P * (4 * ti + s):P * (4 * ti + s) + P],
                                     xnt[s][:, cs],
                                     start=(s == 0), stop=(s == 3))
                nc.scalar.activation(projT[t][:, cs], pt[:],
                                     mybir.ActivationFunctionType.Silu,
                                     bias=small["bU_col"][:, t:t + 1], scale=1.0)
            for r in range(4 * c, 4 * c + 4):
                pt = psum.tile([P, 512], F32, tag="proj", name="ptv")
                nc.tensor.matmul(pt[:], ident[:], bUv16[:], start=True, stop=False)
                for s in range(4):
                    nc.tensor.matmul(pt[:], xnt[s][:, P * r:P * r + P],
                                     w_uvv[:, 512 * s:512 * s + 512],
                                     start=False, stop=(s == 3))
                tmpv = pools.tile([P, D], F16, tag=f"w16{r % 4}", name="tmpv")
                nc.scalar.activation(tmpv[:], pt[:],
                                     mybir.ActivationFunctionType.Silu)
                vraw[r] = tmpv

        def vt_scale(rlo, rhi):
            for r in range(rlo, rhi):
                nc.vector.tensor_scalar(vt[r][:], vraw[r][:],
                                        small["vscale_col"][:, r:r + 1],
                                        None, mybir.AluOpType.mult)

        ln_xnt_half(0)
        proj_half(0)
        ln_xnt_half(1)
        proj_half(1)
        vt_scale(0, 4)
        vt_scale(4, NT)

        # ---- attention: query-column sweeps, head-pair major, key tile r
        # inner; rel-bias is injected into PSUM and the qk matmul accumulates
        # on top; the causal mask is already baked into the bias (-100). ----
        attnT = [io.tile([P, N], F16, tag=f"aT{t}", name=f"aT{t}") for t in range(4)]
        muar = io.tile([P, N], F16, tag="muar")
        rsar = io.tile([P, N], F16, tag="rsar")

        def sweep_pair(c, p):
            rmax = min(NT, 4 * (c + 1))
            pa = psum.tile([P, 512], F32, tag="proj", name="pa")
            for r in range(rmax):
                n0 = max(P * r, 512 * c)
                n1 = 512 * (c + 1)
                w = n1 - n0
                pt = psqk.tile([P, 1024], F32, tag="qk", name="ptq")
                qs = qpool.tile([P, 1024], F16, tag="qs", name="qs")
                for hh in range(2):
                    h = 2 * p + hh
                    qt = projT[8 + h // 2]
                    kt = projT[12 + h // 2]
                    pq = 64 * (h % 2)
                    nc.tensor.matmul(pt[:, 512 * hh:512 * hh + w], ident[:],
                                     acc[r][:, n0 - P * r:n1 - P * r],
                                     start=True, stop=False)
                    nc.tensor.matmul(pt[:, 512 * hh:512 * hh + w],
                                     kt[pq:pq + 64, P * r:P * r + P],
                                     qt[pq:pq + 64, n0:n1],
                                     start=False, stop=True)
                if w == 512:
                    nc.scalar.activation(qs[:], pt[:],
                                         mybir.ActivationFunctionType.Silu)
                else:
                    # one strided activation covers both head-halves
                    pin = pt[:].rearrange("p (two f) -> p two f", two=2)[:, :, 0:w]
                    qout = qs[:].rearrange("p (two f) -> p two f", two=2)[:, :, 0:w]
                    nc.scalar.activation(qout, pin,
                                         mybir.ActivationFunctionType.Silu)
                for hh in range(2):
                    h = 2 * p + hh
                    nc.tensor.matmul(pa[64 * hh:64 * hh + 64, 512 - w:512],
                                     vt[r][:, 64 * h:64 * h + 64],
                                     qs[:, 512 * hh:512 * hh + w],
                                     start=(r == 0), stop=(r == rmax - 1),
                                     skip_group_check=True)
            if c == 0:
                nc.scalar.copy(out=attnT[p][:, 512 * c:512 * c + 512], in_=pa[:])
            else:
                nc.vector.tensor_copy(out=attnT[p][:, 512 * c:512 * c + 512],
                                      in_=pa[:])

        def attn_ln_half(c):
            """LN-a stats + vectors for column half c (after its sweep)."""
            st = psqk.tile([P, 1024], F32, tag="qk", name=f"stats_a{c}")
            sa1 = st[0:1, 0:512]
            sa2 = st[0:1, 512:1024]
            cs = slice(512 * c, 512 * c + 512)
            for p in range(4):
                nc.tensor.matmul(sa1, ones_col[:], attnT[p][:, cs],
                                 start=(p == 0), stop=(p == 3),
                                 skip_group_check=True)
                sqa = kpool.tile([P, 512], F16, tag="kta", name="sqa")
                if c == 0:
                    nc.scalar.activation(sqa[:], attnT[p][:, cs],
                                         mybir.ActivationFunctionType.Square)
                else:
                    nc.vector.tensor_tensor(sqa[:], attnT[p][:, cs],
                                            attnT[p][:, cs],
                                            mybir.AluOpType.mult)
                nc.tensor.matmul(sa2, ones_col[:], sqa[:],
                                 start=(p == 0), stop=(p == 3),
                                 skip_group_check=True)
            mua16, rsa16 = ln_half(sa1, sa2, c, "a")
            for vec, rep in [(mua16, muar), (rsa16, rsar)]:
                ptr = psum.tile([P, 512], F32, tag="proj", name="ptr")
                nc.tensor.matmul(ptr[:], ones_row[:], vec[:],
                                 start=True, stop=True)
                nc.vector.tensor_copy(out=rep[:, cs], in_=ptr[:])

        def half_tail(c):
            """u-product + out-projection + store for column half c."""
            cs = slice(512 * c, 512 * c + 512)
            for s in range(4):
                nc.vector.tensor_tensor(attnT[s][:, cs], attnT[s][:, cs],
                                        muar[:, cs], mybir.AluOpType.subtract)
                nc.vector.tensor_tensor(attnT[s][:, cs], attnT[s][:, cs],
                                        rsar[:, cs], mybir.AluOpType.mult)
                if fold_ga and has_bb:
                    nc.vector.tensor_scalar(attnT[s][:, cs], attnT[s][:, cs],
                                            small["bb_col"][:, s:s + 1], None,
                                            mybir.AluOpType.add)
                elif not fold_ga:
                    nc.vector.tensor_scalar(attnT[s][:, cs], attnT[s][:, cs],
                                            small["ga_col"][:, s:s + 1],
                                            small["bb_col"][:, s:s + 1],
                                            mybir.AluOpType.mult,
                                            mybir.AluOpType.add)
                nc.vector.tensor_tensor(attnT[s][:, cs], attnT[s][:, cs],
                                        projT[s][:, cs], mybir.AluOpType.mult)
            for t in range(4 * c, 4 * c + 4):
                po = psum.tile([P, 512], F32, tag="proj", name="outp")
                for s in range(4):
                    nc.tensor.matmul(po[:], attnT[s][:, P * t:P * t + P],
                                     wo[:, 512 * s:512 * s + 512],
                                     start=(s == 0), stop=(s == 3))
                ot = pools.tile([P, D], F32, tag="w32", name="ot")
                if haspad:
                    nc.vector.scalar_tensor_tensor(
                        ot[:], po[:], small["padout_col"][:, t:t + 1], xtile[t][:],
                        mybir.AluOpType.mult, mybir.AluOpType.add)
                else:
                    nc.vector.tensor_tensor(ot[:], po[:], xtile[t][:],
                                            mybir.AluOpType.add)
                nc.sync.dma_start(out_t[P * t:P * t + P, :], ot[:])

        for p in range(4):
            sweep_pair(0, p)
        attn_ln_half(0)          # closes + frees its psum stats tile quickly
        sweep_pair(1, 0)
        half_tail(0)             # hides behind the rest of the c=1 sweep
        for p in range(1, 4):
            sweep_pair(1, p)
        attn_ln_half(1)
        half_tail(1)

    nc.compile()
    return nc


def _prep_inputs(inputs):
    x = np.asarray(inputs["x"], dtype=np.float32)
    ts = np.asarray(inputs["timestamps"]).astype(np.int64)
    pad = np.asarray(inputs["pad_mask"]).astype(np.float32)
    uvqk = np.asarray(inputs["uvqk"], dtype=np.float32)
    W_o = np.asarray(inputs["W_o"], dtype=np.float32)
    b_o = np.asarray(inputs["b_o"], dtype=np.float32)
    gx = np.asarray(inputs["gamma_x"], dtype=np.float32)
    bx = np.asarray(inputs["beta_x"], dtype=np.float32)
    ga = np.asarray(inputs["gamma_a"], dtype=np.float32)
    ba = np.asarray(inputs["beta_a"], dtype=np.float32)
    ts_w = np.asarray(inputs["ts_w"], dtype=np.float32)
    pos_w = np.asarray(inputs["pos_w"], dtype=np.float32)

    tsq = np.concatenate([ts[:, 1:], ts[:, -1:]], axis=1)  # [B, N]

    uvqk_g = (uvqk * gx[:, None]).astype(np.float32)
    bU = bx @ uvqk  # [E]
    bU_col = bU.reshape(E // P, P).T.copy()  # [P, E//P]
    bUv16 = np.broadcast_to(bU[512:1024], (P, 512)).astype(np.float16)
    ga_col = ga.reshape(4, P).T.copy()

    fold_ga = bool(np.all(np.abs(ga) > 1e-8))
    has_bb = bool(np.any(ba != 0.0))
    haspad = bool(np.any(pad != 0.0))
    W_o_eff = W_o * ga[:, None] if fold_ga else W_o
    ba_eff = (ba / ga) if fold_ga else ba
    ba_col = (ba_eff.reshape(4, P).T.copy()).astype(np.float32)

    uqk_tiles = [0, 1, 2, 3] + list(range(8, 16))
    w_uqk = np.zeros((P, 12 * 4 * P), np.float16)
    for ti, t in enumerate(uqk_tiles):
        for s in range(4):
            w_uqk[:, P * (4 * ti + s):P * (4 * ti + s) + P] = \
                uvqk_g[P * s:P * s + P, P * t:P * t + P]
    w_uvv = np.zeros((P, 4 * 512), np.float16)
    wo_w = np.zeros((P, 4 * 512), np.float16)
    for s in range(4):
        w_uvv[:, 512 * s:512 * s + 512] = uvqk_g[P * s:P * s + P, 512:1024]
        wo_w[:, 512 * s:512 * s + 512] = W_o_eff[P * s:P * s + P, :]

    # full rel-bias tiles in [m, n] layout: pos table + ts_w bucket lookup,
    # causal mask baked in as -100 (silu(-100) == 0 in f16)
    widths = [N - P * r for r in range(NT)]
    offs = np.concatenate([[0], np.cumsum(widths)]).astype(int)
    nidx = np.arange(N)
    tri = np.tril(np.ones((P, P), bool), k=-1)  # m > n within the diag tile
    posacc_all = []
    for b in range(B):
        posacc = np.zeros((P, int(offs[-1])), np.float32)
        for r in range(NT):
            m = P * r + np.arange(P)[:, None]
            nn = nidx[None, P * r:]
            diff = tsq[b][nn] - ts[b][m]
            posacc[:, offs[r]:offs[r + 1]] = (pos_w[nn - m + (N - 1)]
                                              + ts_w[_bucket(diff)])
            blk = posacc[:, offs[r]:offs[r] + P]
            blk[tri] = -100.0
            posacc[:, offs[r]:offs[r] + P] = blk
        posacc_all.append(posacc.astype(np.float16))

    per_core = []
    for b in range(B):
        xr = ((x[b] + b_o[None, :]) * (1.0 - pad[b])[:, None]).astype(np.float16)
        per_core.append({
            "xT16": np.ascontiguousarray(x[b].T.astype(np.float16)),
            "xr": xr,
            "w_uqk": w_uqk, "w_uvv": w_uvv, "wo_w": wo_w,
            "bU_col": bU_col, "bUv16": bUv16,
            "ga_col": ga_col, "bb_col": ba_col,
            "vscale_col": np.ascontiguousarray(
                ((1.0 - pad[b]) / N).astype(np.float32).reshape(NT, P).T),
            "padout_col": np.ascontiguousarray(
                (1.0 - pad[b]).astype(np.float32).reshape(NT, P).T),
            "posacc16": posacc_all[b],
        })
    return per_core, (fold_ga, has_bb, haspad)


def kernel(**inputs):
    from concourse.bass_utils import run_bass_kernel_spmd

    per_core, key = _prep_inputs(inputs)
    if key not in _cache:
        _cache.clear()
        _cache[key] = _build(*key)
    nc = _cache[key]
    res = run_bass_kernel_spmd(nc, per_core, list(range(B)))
    out = np.stack([res.results[b]["out"] for b in range(B)], axis=0)
    return out.astype(np.float32)


# revision 4
# speedup vs baseline: 1.0990x; 1.0990x over previous
"""HSTU block kernel for Trainium2, 8-core data-parallel over batch.

Matmul strategy: fp8e4m3 DoubleRow (2 contraction sub-tiles per matmul, 0.5
PE cycles/output-col) for everything whose operand layout we control — the
uqk/v projections (weights host-packed in [K, 2, M] k-subtile pairs and
pre-scaled by 2^9 so e4m3 stays in its normal range; compensated by the
silu's scale argument), the output projection (normalized-attn*u written
directly as fp8 k-subtile pairs), and the rel-bias PSUM injects (fp8
identity x bias table). qk and attn@v stay f16 (their operands come out of
PSUM in layouts that cannot be repacked for DoubleRow).

The rel-bias (pos table + ts_w bucket lookup, both index-derived tables
expanded on host, causal mask baked in as -100: silu(-100) == 0) is
preloaded into PSUM via the fp8 identity matmul and the qk matmul
accumulates on top. Attention runs as two query-column sweeps, head-pair
major, key tiles innermost, 3-deep PSUM window; each column half finishes
its LN-a stats, u-product, output projection and store tucked behind the
other half's sweep.
"""

import sys

sys.path.insert(0, "/opt/trn_rl_repo")

import numpy as np

import concourse.bass as bass
import concourse.tile as tile
import concourse.mybir as mybir
from concourse import bacc

B, N, D = 8, 1024, 512
H, DV, DQ = 8, 64, 64
E = 2 * H * DV + 2 * H * DQ  # 2048
EPS = 1e-5
P = 128
NT = N // P  # 8 row tiles
F32 = mybir.dt.float32
F16 = mybir.dt.float16
F8 = mybir.dt.float8e4
NP8 = mybir.dt.np(F8)
WS = 2.0 ** 9     # host pre-scale on fp8 weights
WS_INV = 2.0 ** -9
DR = mybir.MatmulPerfMode.DoubleRow
A = mybir.AluOpType

_cache = {}


def _bucket(d):
    d = np.maximum(np.abs(d), 1).astype(np.float32)
    return np.clip((np.log(d) / np.float32(0.301)).astype(np.int32), 0, 128)


def _r2(ap):
    return ap.rearrange("p (two n) -> p two n", two=2)


def _build(fold_ga, has_bb, haspad, has_bu):
    nc = bacc.Bacc()
    d = {}
    for name, shape, dt in [
        ("xT16", [D, N], F16), ("xr", [N, D], F16),
        ("w_uqk8", [P, 12 * 2 * 256], F8),
        ("w_uvv8", [P, 2 * 1024], F8), ("wo8", [P, 2 * 1024], F8),
        ("ident8", [P, 256], F8),
        ("bU_col", [P, E // P], F32), ("bUv512", [P, 2 * DV * H], F8),
        ("ga_col", [P, 4], F32), ("bb_col", [P, 4], F32),
        ("vscale_col", [P, NT], F32), ("padout_col", [P, NT], F32),
        ("acc8", [P, 2 * 4608], F8),
    ]:
        d[name] = nc.dram_tensor(name, shape, dt, kind="ExternalInput")
    out_t = nc.dram_tensor("out", [N, D], F32, kind="ExternalOutput")

    widths = [N - P * r for r in range(NT)]
    offs = np.concatenate([[0], np.cumsum(widths)]).astype(int)
    uqk_tiles = [0, 1, 2, 3] + list(range(8, 16))

    from contextlib import ExitStack
    with tile.TileContext(nc) as tc, ExitStack() as ctx:
        io = ctx.enter_context(tc.tile_pool(name="io", bufs=1))
        pools = ctx.enter_context(tc.tile_pool(name="work", bufs=4))
        kpool = ctx.enter_context(tc.tile_pool(name="kpool", bufs=4))
        qpool = ctx.enter_context(tc.tile_pool(name="qpool", bufs=4))
        psum = ctx.enter_context(tc.tile_pool(name="psum", bufs=2, space="PSUM"))
        psqk = ctx.enter_context(tc.tile_pool(name="psqk", bufs=3, space="PSUM"))

        # ---- persistent SBUF tensors ----
        xT = [io.tile([P, N], F16, tag=f"xT{s}", name=f"xT{s}") for s in range(4)]
        for s in range(4):
            nc.sync.dma_start(xT[s][:], d["xT16"][P * s:P * s + P, :])
        w_uqk8 = io.tile([P, 12 * 2 * 256], F8, tag="w_uqk8")
        for h2 in range(2):
            nc.sync.dma_start(w_uqk8[:, 3072 * h2:3072 * h2 + 3072],
                              d["w_uqk8"][:, 3072 * h2:3072 * h2 + 3072])
        w_uvv8 = io.tile([P, 2 * 1024], F8, tag="w_uvv8")
        nc.sync.dma_start(w_uvv8[:], d["w_uvv8"][:])
        wo8 = io.tile([P, 2 * 1024], F8, tag="wo8")
        nc.sync.dma_start(wo8[:], d["wo8"][:])
        ident8 = io.tile([P, 256], F8, tag="ident8")
        nc.sync.dma_start(ident8[:], d["ident8"][:])
        small = {}
        for nm, sh in [("bU_col", [P, E // P]),
                       ("ga_col", [P, 4]), ("bb_col", [P, 4]),
                       ("vscale_col", [P, NT]), ("padout_col", [P, NT])]:
            small[nm] = io.tile(sh, F32, tag=nm, name=nm)
            nc.sync.dma_start(small[nm][:], d[nm][:])
        bUv512 = io.tile([P, 2 * DV * H], F8, tag="bUv512")
        if has_bu:
            nc.sync.dma_start(bUv512[:], d["bUv512"][:])
        acc = [io.tile([P, 2 * widths[r]], F8, tag=f"acc{r}", name=f"acc{r}")
               for r in range(NT)]
        for r in range(NT):
            nc.sync.dma_start(acc[r][:],
                              d["acc8"][:, 2 * offs[r]:2 * offs[r + 1]])
        xtile = [io.tile([P, D], F16, tag=f"xt{t}", name=f"xt{t}") for t in range(NT)]
        for t in range(NT):
            nc.sync.dma_start(xtile[t][:], d["xr"][P * t:P * t + P, :])

        ones_col = io.tile([P, 1], F16, tag="ones_col")
        nc.vector.memset(ones_col[:], 1.0)
        ones_row = io.tile([1, P], F16, tag="ones_row")
        nc.vector.memset(ones_row[:], 1.0)
        eps_t = io.tile([1, 1], F32, tag="eps_t")
        nc.vector.memset(eps_t[:], EPS)

        # ---- layernorm stats of x: four [1,512] accumulators packed into
        # one 2-bank PSUM tile (partitions 0/32 x column halves) ----
        stats_x = psqk.tile([P, 1024], F32, tag="qk", name="stats_x")
        s1p = [stats_x[32 * c:32 * c + 1, 0:512] for c in range(2)]
        s2p = [stats_x[32 * c:32 * c + 1, 512:1024] for c in range(2)]
        for s in range(4):
            sq = kpool.tile([P, N], F16, tag="kt", name="sq")
            nc.vector.tensor_tensor(sq[:], xT[s][:], xT[s][:], A.mult)
            for c in range(2):
                nc.tensor.matmul(s1p[c], ones_col[:],
                                 xT[s][:, 512 * c:512 * c + 512],
                                 start=(s == 0), stop=(s == 3),
                                 skip_group_check=True)
                nc.tensor.matmul(s2p[c], ones_col[:],
                                 sq[:, 512 * c:512 * c + 512],
                                 start=(s == 0), stop=(s == 3),
                                 skip_group_check=True)

        def ln_half(s1c, s2c, c, tagpfx):
            """mu16, rs16 [1, 512] f16 for column half c from psum sums."""
            mu16 = io.tile([1, 512], F16, tag=f"{tagpfx}mu16_{c}")
            var = pools.tile([1, 512], F32, tag="v32", name="var")
            nc.vector.tensor_scalar_mul(mu16[:], s1c, 1.0 / D)
            mu2 = pools.tile([1, 512], F32, tag="v32", name="mu2")
            nc.vector.tensor_tensor(mu2[:], mu16[:], mu16[:], A.mult)
            nc.vector.scalar_tensor_tensor(var[:], s2c, 1.0 / D, mu2[:],
                                           A.mult, A.subtract)
            nc.scalar.activation(var[:], var[:],
                                 mybir.ActivationFunctionType.Sqrt,
                                 bias=eps_t[:], scale=1.0)
            rs16 = io.tile([1, 512], F16, tag=f"{tagpfx}rs16_{c}")
            with nc.allow_low_precision(reason="rs ~ O(1); f16 rel err ok"):
                nc.vector.reciprocal(rs16[:], var[:])
            return mu16, rs16

        mur = io.tile([P, N], F16, tag="mur")
        rsr = io.tile([P, N], F16, tag="rsr")
        # normalized x in fp8, k-subtile-pair layout: xnt8[j] holds d-chunks
        # (2j, 2j+1) as [P, two, N]
        xnt8 = [io.tile([P, 2 * N], F8, tag=f"xnt8{j}", name=f"xnt8{j}")
                for j in range(2)]
        projT = {}
        for t in uqk_tiles:
            projT[t] = io.tile([P, N], F16, tag=f"pT{t}", name=f"pT{t}")
        vt = [io.tile([P, D], F16, tag=f"v{r}", name=f"v{r}") for r in range(NT)]
        vraw = [None] * NT

        def ln_xnt_half(c):
            mu16, rs16 = ln_half(s1p[c], s2p[c], c, "x")
            nc.gpsimd.partition_broadcast(mur[:, 512 * c:512 * c + 512], mu16[:])
            nc.gpsimd.partition_broadcast(rsr[:, 512 * c:512 * c + 512], rs16[:])
            cs = slice(512 * c, 512 * c + 512)
            for s in range(4):
                t16 = pools.tile([P, 512], F16, tag="xm", name="t16")
                nc.vector.scalar_tensor_tensor(t16[:], xT[s][:, cs], 1.0,
                                               mur[:, cs], A.mult, A.subtract)
                dst = xnt8[s // 2][:, N * (s % 2) + 512 * c:
                                   N * (s % 2) + 512 * c + 512]
                eng = nc.vector if s % 2 == 0 else nc.gpsimd
                eng.tensor_tensor(dst, t16[:], rsr[:, cs], A.mult)

        def proj_half(c):
            """u/q/k projection cols [512c, 512c+512) and v tiles 4c..4c+3."""
            cs = slice(512 * c, 512 * c + 512)
            xv = [_r2(xnt8[j][:])[:, :, cs] for j in range(2)]
            for ti, t in enumerate(uqk_tiles):
                pt = psum.tile([P, 512], F32, tag="proj", name="pt")
                for kp in range(2):
                    w8v = _r2(w_uqk8[:, (ti * 2 + kp) * 256:
                                     (ti * 2 + kp) * 256 + 256])
                    nc.tensor.matmul(pt[:], w8v, xv[kp],
                                     start=(kp == 0), stop=(kp == 1),
                                     perf_mode=DR)
                if has_bu:
                    nc.scalar.activation(projT[t][:, cs], pt[:],
                                         mybir.ActivationFunctionType.Silu,
                                         bias=small["bU_col"][:, t:t + 1],
                                         scale=WS_INV)
                else:
                    nc.scalar.activation(projT[t][:, cs], pt[:],
                                         mybir.ActivationFunctionType.Silu,
                                         scale=WS_INV)
            for r in range(4 * c, 4 * c + 4):
                pt = psum.tile([P, 512], F32, tag="proj", name="ptv")
                if has_bu:
                    nc.tensor.matmul(pt[:], _r2(ident8[:]), _r2(bUv512[:]),
                                     start=True, stop=False, perf_mode=DR)
                for kp in range(2):
                    lv = _r2(xnt8[kp][:])[:, :, P * r:P * r + P]
                    wv = _r2(w_uvv8[:, 1024 * kp:1024 * kp + 1024])
                    nc.tensor.matmul(pt[:], lv, wv,
                                     start=(kp == 0 and not has_bu),
                                     stop=(kp == 1), perf_mode=DR)
                tmpv = pools.tile([P, D], F16, tag=f"w16{r % 4}", name="tmpv")
                nc.scalar.activation(tmpv[:], pt[:],
                                     mybir.ActivationFunctionType.Silu,
                                     scale=WS_INV)
                vraw[r] = tmpv

        def vt_scale(rlo, rhi):
            for r in range(rlo, rhi):
                nc.vector.tensor_scalar(vt[r][:], vraw[r][:],
                                        small["vscale_col"][:, r:r + 1],
                                        None, A.mult)

        ln_xnt_half(0)
        proj_half(0)
        ln_xnt_half(1)
        proj_half(1)
        vt_scale(0, 4)
        vt_scale(4, NT)

        # ---- attention: query-column sweeps, head-pair major, key tile r
        # inner; rel-bias is injected into PSUM (fp8 identity matmul) and the
        # qk matmul accumulates on top; causal mask baked into the bias. ----
        attnT = [io.tile([P, N], F16, tag=f"aT{t}", name=f"aT{t}") for t in range(4)]
        muar = io.tile([P, N], F16, tag="muar")
        rsar = io.tile([P, N], F16, tag="rsar")
        # normalized-attn * u in fp8 k-subtile pairs for the out-projection
        ua8 = [io.tile([P, 2 * N], F8, tag=f"ua8{j}", name=f"ua8{j}")
               for j in range(2)]

        def sweep_pair(c, p):
            rmax = min(NT, 4 * (c + 1))
            pa = psum.tile([P, 512], F32, tag="proj", name="pa")
            for r in range(rmax):
                n0 = max(P * r, 512 * c)
                n1 = 512 * (c + 1)
                w = n1 - n0
                pt = psqk.tile([P, 1024], F32, tag="qk", name="ptq")
                qs = qpool.tile([P, 1024], F16, tag="qs", name="qs")
                accv = _r2(acc[r][:])[:, :, n0 - P * r:n1 - P * r]
                for hh in range(2):
                    h = 2 * p + hh
                    qt = projT[8 + h // 2]
                    kt = projT[12 + h // 2]
                    pq = 64 * (h % 2)
                    nc.tensor.matmul(pt[:, 512 * hh:512 * hh + w],
                                     _r2(ident8[:]), accv,
                                     start=True, stop=False, perf_mode=DR)
                    nc.tensor.matmul(pt[:, 512 * hh:512 * hh + w],
                                     kt[pq:pq + 64, P * r:P * r + P],
                                     qt[pq:pq + 64, n0:n1],
                                     start=False, stop=True)
                if w == 512:
                    nc.scalar.activation(qs[:], pt[:],
                                         mybir.ActivationFunctionType.Silu)
                else:
                    # one strided activation covers both head-halves
                    pin = pt[:].rearrange("p (two f) -> p two f", two=2)[:, :, 0:w]
                    qout = qs[:].rearrange("p (two f) -> p two f", two=2)[:, :, 0:w]
                    nc.scalar.activation(qout, pin,
                                         mybir.ActivationFunctionType.Silu)
                for hh in range(2):
                    h = 2 * p + hh
                    nc.tensor.matmul(pa[64 * hh:64 * hh + 64, 512 - w:512],
                                     vt[r][:, 64 * h:64 * h + 64],
                                     qs[:, 512 * hh:512 * hh + w],
                                     start=(r == 0), stop=(r == rmax - 1),
                                     skip_group_check=True)
            if c == 0:
                nc.scalar.copy(out=attnT[p][:, 512 * c:512 * c + 512], in_=pa[:])
            else:
                nc.vector.tensor_copy(out=attnT[p][:, 512 * c:512 * c + 512],
                                      in_=pa[:])

        def attn_ln_half(c):
            """LN-a stats + vectors for column half c (after its sweep)."""
            st = psqk.tile([P, 1024], F32, tag="qk", name=f"stats_a{c}")
            sa1 = st[0:1, 0:512]
            sa2 = st[0:1, 512:1024]
            cs = slice(512 * c, 512 * c + 512)
            for p in range(4):
                nc.tensor.matmul(sa1, ones_col[:], attnT[p][:, cs],
                                 start=(p == 0), stop=(p == 3),
                                 skip_group_check=True)
                sqa = kpool.tile([P, 512], F16, tag="kta", name="sqa")
                nc.vector.tensor_tensor(sqa[:], attnT[p][:, cs],
                                        attnT[p][:, cs], A.mult)
                nc.tensor.matmul(sa2, ones_col[:], sqa[:],
                                 start=(p == 0), stop=(p == 3),
                                 skip_group_check=True)
            mua16, rsa16 = ln_half(sa1, sa2, c, "a")
            for vec, rep in [(mua16, muar), (rsa16, rsar)]:
                ptr = psum.tile([P, 512], F32, tag="proj", name="ptr")
                nc.tensor.matmul(ptr[:], ones_row[:], vec[:],
                                 start=True, stop=True)
                nc.vector.tensor_copy(out=rep[:, cs], in_=ptr[:])

        def half_tail(c):
            """u-product (into fp8 pairs) + out-projection + store."""
            cs = slice(512 * c, 512 * c + 512)
            for s in range(4):
                a2 = kpool.tile([P, 512], F16, tag="kta", name="a2")
                nc.vector.scalar_tensor_tensor(a2[:], attnT[s][:, cs], 1.0,
                                               muar[:, cs], A.mult, A.subtract)
                if fold_ga and has_bb:
                    nc.vector.tensor_scalar(a2[:], a2[:],
                                            small["bb_col"][:, s:s + 1], None,
                                            A.add)
                elif not fold_ga:
                    nc.vector.tensor_scalar(a2[:], a2[:],
                                            small["ga_col"][:, s:s + 1],
                                            small["bb_col"][:, s:s + 1],
                                            A.mult, A.add)
                nc.vector.scalar_tensor_tensor(a2[:], a2[:], 1.0,
                                               rsar[:, cs], A.mult, A.mult)
                dst = ua8[s // 2][:, N * (s % 2) + 512 * c:
                                  N * (s % 2) + 512 * c + 512]
                nc.vector.scalar_tensor_tensor(dst, a2[:], 1.0,
                                               projT[s][:, cs], A.mult, A.mult)
            for t in range(4 * c, 4 * c + 4):
                po = psum.tile([P, 512], F32, tag="proj", name="outp")
                for kp in range(2):
                    lv = _r2(ua8[kp][:])[:, :, P * t:P * t + P]
                    wv = _r2(wo8[:, 1024 * kp:1024 * kp + 1024])
                    nc.tensor.matmul(po[:], lv, wv,
                                     start=(kp == 0), stop=(kp == 1),
                                     perf_mode=DR)
                ot = pools.tile([P, D], F32, tag="w32", name="ot")
                if haspad:
                    po2 = pools.tile([P, D], F32, tag="w32b", name="po2")
                    nc.vector.tensor_scalar(po2[:], po[:],
                                            small["padout_col"][:, t:t + 1],
                                            WS_INV, A.mult, A.mult)
                    nc.vector.tensor_tensor(ot[:], po2[:], xtile[t][:], A.add)
                else:
                    nc.vector.scalar_tensor_tensor(ot[:], po[:], WS_INV,
                                                   xtile[t][:], A.mult, A.add)
                nc.sync.dma_start(out_t[P * t:P * t + P, :], ot[:])

        for p in range(4):
            sweep_pair(0, p)
        attn_ln_half(0)          # closes + frees its psum stats tile quickly
        sweep_pair(1, 0)
        half_tail(0)             # hides behind the rest of the c=1 sweep
        for p in range(1, 4):
            sweep_pair(1, p)
        attn_ln_half(1)
        half_tail(1)

    nc.compile()
    return nc


def _prep_inputs(inputs):
    x = np.asarray(inputs["x"], dtype=np.float32)
    ts = np.asarray(inputs["timestamps"]).astype(np.int64)
    pad = np.asarray(inputs["pad_mask"]).astype(np.float32)
    uvqk = np.asarray(inputs["uvqk"], dtype=np.float32)
    W_o = np.asarray(inputs["W_o"], dtype=np.float32)
    b_o = np.asarray(inputs["b_o"], dtype=np.float32)
    gx = np.asarray(inputs["gamma_x"], dtype=np.float32)
    bx = np.asarray(inputs["beta_x"], dtype=np.float32)
    ga = np.asarray(inputs["gamma_a"], dtype=np.float32)
    ba = np.asarray(inputs["beta_a"], dtype=np.float32)
    ts_w = np.asarray(inputs["ts_w"], dtype=np.float32)
    pos_w = np.asarray(inputs["pos_w"], dtype=np.float32)

    tsq = np.concatenate([ts[:, 1:], ts[:, -1:]], axis=1)  # [B, N]

    uvqk_g = (uvqk * gx[:, None]).astype(np.float32)
    bU = bx @ uvqk  # [E]
    bU_col = bU.reshape(E // P, P).T.copy()  # [P, E//P]
    ga_col = ga.reshape(4, P).T.copy()

    fold_ga = bool(np.all(np.abs(ga) > 1e-8))
    has_bb = bool(np.any(ba != 0.0))
    haspad = bool(np.any(pad != 0.0))
    has_bu = bool(np.any(bU != 0.0))
    W_o_eff = W_o * ga[:, None] if fold_ga else W_o
    ba_eff = (ba / ga) if fold_ga else ba
    ba_col = (ba_eff.reshape(4, P).T.copy()).astype(np.float32)

    # fp8 weights: k-subtile-pair layout [P, 2, M], pre-scaled by WS=2^9 so
    # e4m3 stays in normal range; the silu compensates with scale=2^-9.
    uqk_tiles = [0, 1, 2, 3] + list(range(8, 16))
    w_uqk8 = np.zeros((P, 12 * 2 * 256), NP8)
    for ti, t in enumerate(uqk_tiles):
        for kp in range(2):
            blk = np.zeros((P, 2, P), np.float32)
            for j in range(2):
                s = 2 * kp + j
                blk[:, j, :] = uvqk_g[P * s:P * s + P, P * t:P * t + P] * WS
            w_uqk8[:, (ti * 2 + kp) * 256:(ti * 2 + kp) * 256 + 256] = \
                blk.reshape(P, 256).astype(NP8)
    w_uvv8 = np.zeros((P, 2 * 1024), NP8)
    wo8 = np.zeros((P, 2 * 1024), NP8)
    for kp in range(2):
        blkv = np.zeros((P, 2, 512), np.float32)
        blko = np.zeros((P, 2, 512), np.float32)
        for j in range(2):
            s = 2 * kp + j
            blkv[:, j, :] = uvqk_g[P * s:P * s + P, 512:1024] * WS
            blko[:, j, :] = W_o_eff[P * s:P * s + P, :] * WS
        w_uvv8[:, 1024 * kp:1024 * kp + 1024] = blkv.reshape(P, 1024).astype(NP8)
        wo8[:, 1024 * kp:1024 * kp + 1024] = blko.reshape(P, 1024).astype(NP8)
    ident8 = np.zeros((P, 2, P), NP8)
    ident8[:, 0, :] = np.eye(P, dtype=NP8)
    ident8 = ident8.reshape(P, 256)
    bUv512 = np.broadcast_to(bU[512:1024] * WS, (P, 512)).astype(NP8)
    bUv512 = np.concatenate([bUv512, np.zeros((P, 512), NP8)], axis=1)

    # full rel-bias tiles in [m, n] layout: pos table + ts_w bucket lookup,
    # causal mask baked in as -100; fp8 pairs with a zero second k-subtile
    # for the DoubleRow identity inject.
    widths = [N - P * r for r in range(NT)]
    offs = np.concatenate([[0], np.cumsum(widths)]).astype(int)
    nidx = np.arange(N)
    tri = np.tril(np.ones((P, P), bool), k=-1)  # m > n within the diag tile
    acc8_all = []
    for b in range(B):
        acc8 = np.zeros((P, 2 * int(offs[-1])), NP8)
        for r in range(NT):
            m = P * r + np.arange(P)[:, None]
            nn = nidx[None, P * r:]
            diff = tsq[b][nn] - ts[b][m]
            blk = (pos_w[nn - m + (N - 1)] + ts_w[_bucket(diff)])
            blk[:, 0:P][tri] = -96.0
            w2 = np.zeros((P, 2, widths[r]), np.float32)
            w2[:, 0, :] = blk
            acc8[:, 2 * offs[r]:2 * offs[r + 1]] = \
                w2.reshape(P, 2 * widths[r]).astype(NP8)
        acc8_all.append(acc8)

    per_core = []
    for b in range(B):
        xr = ((x[b] + b_o[None, :]) * (1.0 - pad[b])[:, None]).astype(np.float16)
        per_core.append({
            "xT16": np.ascontiguousarray(x[b].T.astype(np.float16)),
            "xr": xr,
            "w_uqk8": w_uqk8, "w_uvv8": w_uvv8, "wo8": wo8, "ident8": ident8,
            "bU_col": bU_col, "bUv512": bUv512,
            "ga_col": ga_col, "bb_col": ba_col,
            "vscale_col": np.ascontiguousarray(
                ((1.0 - pad[b]) / N).astype(np.float32).reshape(NT, P).T),
            "padout_col": np.ascontiguousarray(
                (1.0 - pad[b]).astype(np.float32).reshape(NT, P).T),
            "acc8": acc8_all[b],
        })
    return per_core, (fold_ga, has_bb, haspad, has_bu)


def kernel(**inputs):
    from concourse.bass_utils import run_bass_kernel_spmd

    per_core, key = _prep_inputs(inputs)
    if key not in _cache:
        _cache.clear()
        _cache[key] = _build(*key)
    nc = _cache[key]
    res = run_bass_kernel_spmd(nc, per_core, list(range(B)))
    out = np.stack([res.results[b]["out"] for b in range(B)], axis=0)
    return out.astype(np.float32)


# revision 70
# speedup vs baseline: 1.2721x; 1.1575x over previous
"""HSTU block kernel for Trainium2, 8-core data-parallel over batch.

Matmul strategy: fp8e4m3 DoubleRow (2 contraction sub-tiles per matmul, 0.5
PE cycles/output-col) for everything whose operand layout we control — the
uqk/v projections (weights host-packed in [K, 2, M] k-subtile pairs and
pre-scaled by 2^9 so e4m3 stays in its normal range; compensated by the
silu's scale argument), the output projection (normalized-attn*u written
directly as fp8 k-subtile pairs), and the rel-bias PSUM injects (fp8
identity x bias table). qk and attn@v stay f16 (their operands come out of
PSUM in layouts that cannot be repacked for DoubleRow).

LayerNorm sites avoid the Activation engine entirely (it runs silu only,
one act-table load for the whole kernel): column stats land in PSUM rows,
a DVE 32x32 stream-transpose turns them into a [32, 16]-strided layout
where the whole mean/var/rsqrt chain runs at free-size 16 (rsqrt = one
Newton step off a linear seed for x-LN where var~1, or the quake bit-hack
seed for attn-LN), and per-quantity back-transposes lay mu/rs out as rows
for the Pool partition-broadcast — mu early so the subtract passes overlap
the rsqrt chain.

The rel-bias (pos table + ts_w bucket lookup, both index-derived tables
expanded on host, causal mask baked in as -96: silu(-96) == 0) is
preloaded into PSUM via the fp8 identity matmul and the qk matmul
accumulates on top. Attention runs as two query-column sweeps, head-pair
major, key tiles innermost, 3-deep PSUM window; each column half finishes
its LN-a stats, u-product, output projection and store tucked behind the
other half's sweep.
"""

import sys

sys.path.insert(0, "/opt/trn_rl_repo")

import numpy as np

import concourse.bass as bass
import concourse.tile as tile
import concourse.mybir as mybir
from concourse import bacc

B, N, D = 8, 1024, 512
H, DV, DQ = 8, 64, 64
E = 2 * H * DV + 2 * H * DQ  # 2048
EPS = 1e-5
P = 128
NT = N // P  # 8 row tiles
F32 = mybir.dt.float32
F16 = mybir.dt.float16
F8 = mybir.dt.float8e4
I32 = mybir.dt.int32
NP8 = mybir.dt.np(F8)
WS = 2.0 ** 9     # host pre-scale on fp8 weights
WS_INV = 2.0 ** -9
DR = mybir.MatmulPerfMode.DoubleRow
A = mybir.AluOpType
MAGIC = 0x5F3759DF

_cache = {}


def _bucket(d):
    d = np.maximum(np.abs(d), 1).astype(np.float32)
    return np.clip((np.log(d) / np.float32(0.301)).astype(np.int32), 0, 128)


def _r2(ap):
    return ap.rearrange("p (two n) -> p two n", two=2)


def _build(fold_ga, has_bb, haspad, has_bu):
    nc = bacc.Bacc()
    d = {}
    for name, shape, dt in [
        ("xT16", [D, N], F16), ("xr", [N, D], F16),
        ("w_uqk8", [P, 12 * 2 * 256], F8),
        ("w_uvv8", [P, 2 * 1024], F8), ("wo8", [P, 2 * 1024], F8),
        ("ident8", [P, 256], F8),
        ("bU_col", [P, E // P], F32), ("bUv512", [P, 2 * DV * H], F8),
        ("ga_col", [P, 4], F32), ("bb_col", [P, 4], F32),
        ("vscale_col", [P, NT], F32), ("padout_col", [P, NT], F32),
        ("acc8", [P, 2 * 4608], F8),
    ]:
        d[name] = nc.dram_tensor(name, shape, dt, kind="ExternalInput")
    out_t = nc.dram_tensor("out", [N, D], F16, kind="ExternalOutput")

    widths = [N - P * r for r in range(NT)]
    offs = np.concatenate([[0], np.cumsum(widths)]).astype(int)
    uqk_tiles = [0, 1, 2, 3] + list(range(8, 16))

    from contextlib import ExitStack
    with tile.TileContext(nc) as tc, ExitStack() as ctx:
        io = ctx.enter_context(tc.tile_pool(name="io", bufs=1))
        pools = ctx.enter_context(tc.tile_pool(name="work", bufs=4))
        kpool = ctx.enter_context(tc.tile_pool(name="kpool", bufs=4))
        qpool = ctx.enter_context(tc.tile_pool(name="qpool", bufs=4))
        spool = ctx.enter_context(tc.tile_pool(name="spool", bufs=6))
        psum = ctx.enter_context(tc.tile_pool(name="psum", bufs=2, space="PSUM"))
        psqk = ctx.enter_context(tc.tile_pool(name="psqk", bufs=3, space="PSUM"))

        # ---- persistent SBUF tensors ----
        xT = [io.tile([P, N], F16, tag=f"xT{s}", name=f"xT{s}") for s in range(4)]
        for s in range(4):
            nc.sync.dma_start(xT[s][:], d["xT16"][P * s:P * s + P, :])
        w_uqk8 = io.tile([P, 12 * 2 * 256], F8, tag="w_uqk8")
        for h2 in range(2):
            nc.sync.dma_start(w_uqk8[:, 3072 * h2:3072 * h2 + 3072],
                              d["w_uqk8"][:, 3072 * h2:3072 * h2 + 3072])
        w_uvv8 = io.tile([P, 2 * 1024], F8, tag="w_uvv8")
        nc.sync.dma_start(w_uvv8[:], d["w_uvv8"][:])
        wo8 = io.tile([P, 2 * 1024], F8, tag="wo8")
        nc.sync.dma_start(wo8[:], d["wo8"][:])
        ident8 = io.tile([P, 256], F8, tag="ident8")
        nc.sync.dma_start(ident8[:], d["ident8"][:])
        small = {}
        for nm, sh in [("bU_col", [P, E // P]),
                       ("ga_col", [P, 4]), ("bb_col", [P, 4]),
                       ("vscale_col", [P, NT]), ("padout_col", [P, NT])]:
            small[nm] = io.tile(sh, F32, tag=nm, name=nm)
            nc.sync.dma_start(small[nm][:], d[nm][:])
        bUv512 = io.tile([P, 2 * DV * H], F8, tag="bUv512")
        if has_bu:
            nc.sync.dma_start(bUv512[:], d["bUv512"][:])
        acc = [io.tile([P, 2 * widths[r]], F8, tag=f"acc{r}", name=f"acc{r}")
               for r in range(NT)]
        for r in range(NT):
            nc.sync.dma_start(acc[r][:],
                              d["acc8"][:, 2 * offs[r]:2 * offs[r + 1]])
        xtile = [io.tile([P, D], F16, tag=f"xt{t}", name=f"xt{t}") for t in range(NT)]
        for t in range(NT):
            nc.sync.dma_start(xtile[t][:], d["xr"][P * t:P * t + P, :])

        ones_col = io.tile([P, 1], F16, tag="ones_col")
        nc.vector.memset(ones_col[:], 1.0)


        # ---- LN machinery: stats psum tile [P, 1024] (s1 in cols 0:512,
        # s2 in cols 512:1024, both on partition 0) -> stream-transpose to
        # [32, 1024] -> chain at free-size 16 -> [32, 512] M tile with mu on
        # block-col 0, rs on block-col 1 -> transpose back -> rows 0/1 ->
        # Pool partition-broadcast. rsqrt = quake seed + 2 Newton iterations.
        ln_scr = []
        for g in range(2):
            scr = {
                "T": io.tile([32, 1024], F32, tag=f"ln_T{g}", name=f"ln_T{g}"),
                "M": io.tile([32, 1024], F16, tag=f"ln_M{g}", name=f"ln_M{g}"),
                "Tb": io.tile([32, 1024], F16, tag=f"ln_Tb{g}", name=f"ln_Tb{g}"),
                "w1": io.tile([32, 16], F32, tag=f"w1{g}", name=f"w1{g}"),
                "w2": io.tile([32, 16], F32, tag=f"w2{g}", name=f"w2{g}"),
                "w5": io.tile([32, 16], F32, tag=f"w5{g}", name=f"w5{g}"),
                "w6": io.tile([32, 16], F32, tag=f"w6{g}", name=f"w6{g}"),
                "w7": io.tile([32, 16], F32, tag=f"w7{g}", name=f"w7{g}"),
                "i2": io.tile([32, 16], I32, tag=f"i2{g}", name=f"i2{g}"),
            }
            nc.gpsimd.memset(scr["M"][:], 0.0)
            ln_scr.append(scr)

        def strided16(ap_2d, base):
            # cols {base + 32*j : j in 0..15} of a [32, X] AP
            return ap_2d.rearrange("p (j k) -> p j k", k=32)[:, :, base:base + 1]

        def ln_site(st1, st2, mu_dst, rs_dst, cs, g, lin_seed=False):
            """st1/st2: [32,512] psum stat regions (sums / sums of squares).
            Writes broadcast mu/rs to mu_dst[:, cs], rs_dst[:, cs]."""
            sc = ln_scr[g]
            T, M, Tb = sc["T"], sc["M"], sc["Tb"]
            w1, w2, w5, w6, w7, i2 = (sc["w1"], sc["w2"], sc["w5"],
                                      sc["w6"], sc["w7"], sc["i2"])
            nc.vector.transpose(T[:, 0:512], st1)
            s1 = strided16(T[:, 0:512], 0)
            s2 = strided16(T[:, 512:1024], 0)
            mu = strided16(M[:, 0:512], 0)
            rs = strided16(M[:, 512:1024], 0)
            nc.vector.tensor_scalar(mu, s1, 1.0 / D, None, A.mult)
            nc.vector.transpose(Tb[:, 0:512], M[:, 0:512])
            nc.gpsimd.partition_broadcast(mu_dst[:, cs], Tb[0:1, 0:512])
            nc.vector.transpose(T[:, 512:1024], st2)
            nc.vector.tensor_tensor(w1[:], mu, mu, A.mult)
            nc.vector.scalar_tensor_tensor(w2[:], s2, 1.0 / D, w1[:],
                                           A.mult, A.subtract)
            nc.vector.tensor_scalar(w2[:], w2[:], EPS, None, A.add)
            if lin_seed:
                # x-LN: var ~ 1 +- 6%, linear seed beats the quake hack
                nc.vector.tensor_scalar(w7[:], w2[:], -0.5, 1.5, A.mult, A.add)
                y0 = w7[:]
            else:
                # quake rsqrt seed (one Newton iteration below)
                nc.vector.tensor_scalar(i2[:], w2[:].bitcast(I32), 1, None,
                                        A.logical_shift_right)
                nc.vector.tensor_scalar(i2[:], i2[:], -1, MAGIC, A.mult,
                                        A.add)
                y0 = i2[:].bitcast(F32)
            nc.vector.tensor_tensor(w5[:], y0, y0, A.mult)
            nc.vector.tensor_tensor(w6[:], w5[:], w2[:], A.mult)
            nc.vector.tensor_scalar(w6[:], w6[:], -0.5, 1.5, A.mult, A.add)
            nc.vector.tensor_tensor(rs, y0, w6[:], A.mult)
            nc.vector.transpose(Tb[:, 512:1024], M[:, 512:1024])
            nc.gpsimd.partition_broadcast(rs_dst[:, cs], Tb[0:1, 512:1024])

        # ---- x layernorm stats (emitted per half so half 1 cannot be
        # hoisted into half 0's critical path) ----
        st_s1 = psqk.tile([P, 1024], F32, tag="qk", name="st_s1")
        st_s2 = psqk.tile([P, 1024], F32, tag="qk", name="st_s2")
        for s in range(4):
            for c in range(2):
                nc.tensor.matmul(st_s1[0:1, 512 * c:512 * c + 512], ones_col[:],
                                 xT[s][:, 512 * c:512 * c + 512],
                                 start=(s == 0), stop=(s == 3),
                                 skip_group_check=True)
        for s in range(4):
            sq = kpool.tile([P, N], F16, tag="kt", name="sq")
            nc.vector.tensor_tensor(sq[:], xT[s][:], xT[s][:], A.mult)
            for c in range(2):
                nc.tensor.matmul(st_s2[0:1, 512 * c:512 * c + 512], ones_col[:],
                                 sq[:, 512 * c:512 * c + 512],
                                 start=(s == 0), stop=(s == 3),
                                 skip_group_check=True)

        mur = io.tile([P, N], F16, tag="mur")
        rsr = io.tile([P, N], F16, tag="rsr")
        # normalized x in fp8, k-subtile-pair layout: xnt8[j] holds d-chunks
        # (2j, 2j+1) as [P, two, N]
        xnt8 = [io.tile([P, 2 * N], F8, tag=f"xnt8{j}", name=f"xnt8{j}")
                for j in range(2)]
        # u/q/k proj in pair tiles: pair pr holds e-tiles (2pr, 2pr+1) at
        # col blocks [0:1024), [1024:2048)
        projP = [io.tile([P, 2048], F16, tag=f"pP{pr}", name=f"pP{pr}")
                 for pr in range(6)]

        def pj(t):
            """(pair tile, col offset) for logical uqk tile index."""
            ti = uqk_tiles.index(t)
            return projP[ti // 2], 1024 * (ti % 2)

        vt = [io.tile([P, D], F16, tag=f"v{r}", name=f"v{r}") for r in range(NT)]
        vraw2 = [None] * 4

        def ln_xnt_half(c):
            cs = slice(512 * c, 512 * c + 512)
            ln_site(st_s1[0:32, 512 * c:512 * c + 512],
                    st_s2[0:32, 512 * c:512 * c + 512], mur, rsr, cs, 0,
                    lin_seed=True)
            for s in range(4):
                t16 = pools.tile([P, 512], F16, tag="xm", name="t16")
                eng = nc.gpsimd if s == 3 else nc.vector
                eng.tensor_tensor(t16[:], xT[s][:, cs], mur[:, cs],
                                  A.subtract)
                dst = xnt8[s // 2][:, N * (s % 2) + 512 * c:
                                   N * (s % 2) + 512 * c + 512]
                eng.tensor_tensor(dst, t16[:], rsr[:, cs], A.mult)

        projqkv = [None, None]

        def proj_u(c):
            for pr in (0, 1):
                projqkv[c](pr)

        def proj_half(c):
            """u/q/k projection cols [512c, 512c+512) and v tiles 4c..4c+3."""
            cs = slice(512 * c, 512 * c + 512)
            xv = [_r2(xnt8[j][:])[:, :, cs] for j in range(2)]
            def uqk_pair(pr):
                pt = psqk.tile([P, 1024], F32, tag="qk", name="ptp")
                for half in range(2):
                    ti = 2 * pr + half
                    for kp in range(2):
                        w8v = _r2(w_uqk8[:, (ti * 2 + kp) * 256:
                                         (ti * 2 + kp) * 256 + 256])
                        nc.tensor.matmul(pt[:, 512 * half:512 * half + 512],
                                         w8v, xv[kp],
                                         start=(kp == 0), stop=(kp == 1),
                                         perf_mode=DR)
                if has_bu:
                    # per-tile silu: the two e-tiles have different biases
                    for half in range(2):
                        t = uqk_tiles[2 * pr + half]
                        nc.scalar.activation(
                            projP[pr][:, 1024 * half + 512 * c:
                                      1024 * half + 512 * c + 512],
                            pt[:, 512 * half:512 * half + 512],
                            mybir.ActivationFunctionType.Silu,
                            bias=small["bU_col"][:, t:t + 1], scale=WS_INV)
                else:
                    # one strided silu covers both e-tiles of the pair
                    po = projP[pr][:].rearrange("p (b f) -> p b f",
                                                b=2)[:, :, 512 * c:512 * c + 512]
                    pi = pt[:].rearrange("p (b f) -> p b f", b=2)
                    nc.scalar.activation(po, pi,
                                         mybir.ActivationFunctionType.Silu,
                                         scale=WS_INV)
            projqkv[c] = uqk_pair
            for pr in (2, 4):
                uqk_pair(pr)
            for vp in range(2):
                pt = psqk.tile([P, 1024], F32, tag="qk", name="ptv")
                for half in range(2):
                    r = 4 * c + 2 * vp + half
                    ps = pt[:, 512 * half:512 * half + 512]
                    if has_bu:
                        nc.tensor.matmul(ps, _r2(ident8[:]), _r2(bUv512[:]),
                                         start=True, stop=False, perf_mode=DR)
                    for kp in range(2):
                        lv = _r2(xnt8[kp][:])[:, :, P * r:P * r + P]
                        wv = _r2(w_uvv8[:, 1024 * kp:1024 * kp + 1024])
                        nc.tensor.matmul(ps, lv, wv,
                                         start=(kp == 0 and not has_bu),
                                         stop=(kp == 1), perf_mode=DR)
                tmpv = pools.tile([P, 1024], F16, tag=f"w16{vp}", name="tmpv")
                nc.scalar.activation(tmpv[:], pt[:],
                                     mybir.ActivationFunctionType.Silu,
                                     scale=WS_INV)
                vraw2[2 * c + vp] = tmpv
            vt_scale(c)
            for pr in (3, 5):
                uqk_pair(pr)

        def vt_scale(c):
            for vp in range(2):
                for half in range(2):
                    r = 4 * c + 2 * vp + half
                    nc.vector.tensor_scalar(
                        vt[r][:], vraw2[2 * c + vp][:, 512 * half:512 * half + 512],
                        small["vscale_col"][:, r:r + 1], None, A.mult)

        ln_xnt_half(0)
        proj_half(0)
        proj_u(0)

        # ---- attention ----
        attnT = [io.tile([P, N], F16, tag=f"aT{t}", name=f"aT{t}") for t in range(4)]
        muar = io.tile([P, N], F16, tag="muar")
        rsar = io.tile([P, N], F16, tag="rsar")
        # normalized-attn * u in fp8 k-subtile pairs for the out-projection
        ua8 = [io.tile([P, 2 * N], F8, tag=f"ua8{j}", name=f"ua8{j}")
               for j in range(2)]
        sqa_t = {}

        pa_t = {}

        def sweep_pair(c, p):
            rmax = min(NT, 4 * (c + 1))
            pa = psum.tile([P, 512], F32, tag="proj", name="pa")
            pa_t[(c, p)] = pa
            for r in range(rmax):
                n0 = max(P * r, 512 * c)
                n1 = 512 * (c + 1)
                w = n1 - n0
                pt = psqk.tile([P, 1024], F32, tag="qk", name="ptq")
                qs = qpool.tile([P, 1024], F16, tag="qs", name="qs")
                accv = _r2(acc[r][:])[:, :, n0 - P * r:n1 - P * r]
                for hh in range(2):
                    h = 2 * p + hh
                    qtile, qoff = pj(uqk_tiles[4 + h // 2])
                    ktile, koff = pj(uqk_tiles[8 + h // 2])
                    pq = 64 * (h % 2)
                    nc.tensor.matmul(pt[:, 512 * hh:512 * hh + w],
                                     _r2(ident8[:]), accv,
                                     start=True, stop=False, perf_mode=DR)
                    nc.tensor.matmul(pt[:, 512 * hh:512 * hh + w],
                                     ktile[pq:pq + 64, koff + P * r:koff + P * r + P],
                                     qtile[pq:pq + 64, qoff + n0:qoff + n1],
                                     start=False, stop=True)
                if w == 512:
                    nc.scalar.activation(qs[:], pt[:],
                                         mybir.ActivationFunctionType.Silu)
                else:
                    pin = pt[:].rearrange("p (two f) -> p two f", two=2)[:, :, 0:w]
                    qout = qs[:].rearrange("p (two f) -> p two f", two=2)[:, :, 0:w]
                    nc.scalar.activation(qout, pin,
                                         mybir.ActivationFunctionType.Silu)
                for hh in range(2):
                    h = 2 * p + hh
                    nc.tensor.matmul(pa[64 * hh:64 * hh + 64, 512 - w:512],
                                     vt[r][:, 64 * h:64 * h + 64],
                                     qs[:, 512 * hh:512 * hh + w],
                                     start=(r == 0), stop=(r == rmax - 1),
                                     skip_group_check=True)
            nc.vector.tensor_copy(out=attnT[p][:, 512 * c:512 * c + 512],
                                  in_=pa[:])

        def sqa_prep(c, p):
            """square of pair p's attnT columns, ahead of the stats matmuls.
            The last pair squares from PSUM on the (tail-idle) Act engine so
            the DVE spine only carries the psum->sbuf copy."""
            cs = slice(512 * c, 512 * c + 512)
            sqa = spool.tile([P, 512], F16, tag="sqa", name="sqa")
            nc.vector.tensor_tensor(sqa[:], attnT[p][:, cs],
                                    attnT[p][:, cs], A.mult)
            sqa_t[(c, p)] = sqa

        stats_a = [None, None]

        def a_stats(c, plo, phi):
            cs = slice(512 * c, 512 * c + 512)
            if stats_a[c] is None:
                stats_a[c] = psqk.tile([P, 1024], F32, tag="qk",
                                       name=f"stats_a{c}")
            st = stats_a[c]
            for p in range(plo, phi):
                nc.tensor.matmul(st[0:1, 0:512], ones_col[:], attnT[p][:, cs],
                                 start=(p == 0), stop=(p == 3),
                                 skip_group_check=True)
                nc.tensor.matmul(st[0:1, 512:1024], ones_col[:],
                                 sqa_t[(c, p)][:],
                                 start=(p == 0), stop=(p == 3),
                                 skip_group_check=True)

        def tail_ln(c):
            """LN-a + normalized-attn * u into fp8 pairs."""
            cs = slice(512 * c, 512 * c + 512)
            ln_site(stats_a[c][0:32, 0:512], stats_a[c][0:32, 512:1024],
                    muar, rsar, cs, 1)
            for s in range(4):
                a2 = spool.tile([P, 512], F16, tag="sqa", name="a2")
                nc.vector.tensor_tensor(a2[:], attnT[s][:, cs], muar[:, cs],
                                        A.subtract)
                if fold_ga and has_bb:
                    nc.vector.tensor_scalar(a2[:], a2[:],
                                            small["bb_col"][:, s:s + 1], None,
                                            A.add)
                elif not fold_ga:
                    nc.vector.tensor_scalar(a2[:], a2[:],
                                            small["ga_col"][:, s:s + 1],
                                            small["bb_col"][:, s:s + 1],
                                            A.mult, A.add)
                nc.vector.tensor_tensor(a2[:], a2[:], rsar[:, cs], A.mult)
                ptile, poff = pj(uqk_tiles[s])
                dst = ua8[s // 2][:, N * (s % 2) + 512 * c:
                                  N * (s % 2) + 512 * c + 512]
                eng = nc.gpsimd if s % 2 == 1 else nc.vector
                eng.tensor_tensor(dst, a2[:],
                                  ptile[:, poff + 512 * c:
                                        poff + 512 * c + 512],
                                  A.mult)

        def tail_out(c):
            """out-projection + residual + store for column half c. The
            second half's residual rides the PSUM as a f16 identity inject
            (x pre-scaled by 2^9 on host to match the fp8 weight scale) and
            the psum->sbuf move runs on the tail-idle Act engine."""
            for t in range(4 * c, 4 * c + 4):
                po = psum.tile([P, 512], F32, tag="proj", name="outp")
                for kp in range(2):
                    lv = _r2(ua8[kp][:])[:, :, P * t:P * t + P]
                    wv = _r2(wo8[:, 1024 * kp:1024 * kp + 1024])
                    nc.tensor.matmul(po[:], lv, wv,
                                     start=(kp == 0), stop=(kp == 1),
                                     perf_mode=DR)
                ot = pools.tile([P, D], F16, tag="w32", name="ot")
                if haspad:
                    po2 = pools.tile([P, D], F32, tag="w32b", name="po2")
                    nc.vector.tensor_scalar(po2[:], po[:],
                                            small["padout_col"][:, t:t + 1],
                                            WS_INV, A.mult, A.mult)
                    nc.vector.tensor_tensor(ot[:], po2[:], xtile[t][:], A.add)
                else:
                    nc.vector.scalar_tensor_tensor(ot[:], po[:], WS_INV,
                                                   xtile[t][:], A.mult, A.add)
                nc.sync.dma_start(out_t[P * t:P * t + P, :], ot[:])

        sweep_pair(0, 0)
        sqa_prep(0, 0)
        ln_xnt_half(1)
        proj_half(1)
        sweep_pair(0, 1)
        sqa_prep(0, 1)
        proj_u(1)
        for p in range(2, 4):
            sweep_pair(0, p)
            sqa_prep(0, p)
        a_stats(0, 0, 4)
        tail_ln(0)               # hides behind the rest of the c=1 sweep
        sweep_pair(1, 0)
        sqa_prep(1, 0)
        sweep_pair(1, 1)
        sqa_prep(1, 1)
        tail_out(0)
        sweep_pair(1, 2)
        sqa_prep(1, 2)
        sweep_pair(1, 3)
        a_stats(1, 0, 3)
        sqa_prep(1, 3)
        a_stats(1, 3, 4)
        tail_ln(1)
        tail_out(1)

    nc.compile()
    return nc


def _prep_inputs(inputs):
    x = np.asarray(inputs["x"], dtype=np.float32)
    ts = np.asarray(inputs["timestamps"]).astype(np.int64)
    pad = np.asarray(inputs["pad_mask"]).astype(np.float32)
    uvqk = np.asarray(inputs["uvqk"], dtype=np.float32)
    W_o = np.asarray(inputs["W_o"], dtype=np.float32)
    b_o = np.asarray(inputs["b_o"], dtype=np.float32)
    gx = np.asarray(inputs["gamma_x"], dtype=np.float32)
    bx = np.asarray(inputs["beta_x"], dtype=np.float32)
    ga = np.asarray(inputs["gamma_a"], dtype=np.float32)
    ba = np.asarray(inputs["beta_a"], dtype=np.float32)
    ts_w = np.asarray(inputs["ts_w"], dtype=np.float32)
    pos_w = np.asarray(inputs["pos_w"], dtype=np.float32)

    tsq = np.concatenate([ts[:, 1:], ts[:, -1:]], axis=1)  # [B, N]

    uvqk_g = (uvqk * gx[:, None]).astype(np.float32)
    bU = bx @ uvqk  # [E]
    bU_col = bU.reshape(E // P, P).T.copy()  # [P, E//P]
    ga_col = ga.reshape(4, P).T.copy()

    fold_ga = bool(np.all(np.abs(ga) > 1e-8))
    has_bb = bool(np.any(ba != 0.0))
    haspad = bool(np.any(pad != 0.0))
    has_bu = bool(np.any(bU != 0.0))
    W_o_eff = W_o * ga[:, None] if fold_ga else W_o
    ba_eff = (ba / ga) if fold_ga else ba
    ba_col = (ba_eff.reshape(4, P).T.copy()).astype(np.float32)

    # fp8 weights: k-subtile-pair layout [P, 2, M], pre-scaled by WS=2^9 so
    # e4m3 stays in normal range; the silu compensates with scale=2^-9.
    uqk_tiles = [0, 1, 2, 3] + list(range(8, 16))
    w_uqk8 = np.zeros((P, 12 * 2 * 256), NP8)
    for ti, t in enumerate(uqk_tiles):
        for kp in range(2):
            blk = np.zeros((P, 2, P), np.float32)
            for j in range(2):
                s = 2 * kp + j
                blk[:, j, :] = uvqk_g[P * s:P * s + P, P * t:P * t + P] * WS
            w_uqk8[:, (ti * 2 + kp) * 256:(ti * 2 + kp) * 256 + 256] = \
                blk.reshape(P, 256).astype(NP8)
    w_uvv8 = np.zeros((P, 2 * 1024), NP8)
    wo8 = np.zeros((P, 2 * 1024), NP8)
    for kp in range(2):
        blkv = np.zeros((P, 2, 512), np.float32)
        blko = np.zeros((P, 2, 512), np.float32)
        for j in range(2):
            s = 2 * kp + j
            blkv[:, j, :] = uvqk_g[P * s:P * s + P, 512:1024] * WS
            blko[:, j, :] = W_o_eff[P * s:P * s + P, :] * WS
        w_uvv8[:, 1024 * kp:1024 * kp + 1024] = blkv.reshape(P, 1024).astype(NP8)
        wo8[:, 1024 * kp:1024 * kp + 1024] = blko.reshape(P, 1024).astype(NP8)
    ident8 = np.zeros((P, 2, P), NP8)
    ident8[:, 0, :] = np.eye(P, dtype=NP8)
    ident8 = ident8.reshape(P, 256)
    bUv512 = np.broadcast_to(bU[512:1024] * WS, (P, 512)).astype(NP8)
    bUv512 = np.concatenate([bUv512, np.zeros((P, 512), NP8)], axis=1)

    # full rel-bias tiles in [m, n] layout: pos table + ts_w bucket lookup,
    # causal mask baked in as -96; fp8 pairs with a zero second k-subtile
    # for the DoubleRow identity inject.
    widths = [N - P * r for r in range(NT)]
    offs = np.concatenate([[0], np.cumsum(widths)]).astype(int)
    nidx = np.arange(N)
    tri = np.tril(np.ones((P, P), bool), k=-1)  # m > n within the diag tile
    acc8_all = []
    for b in range(B):
        acc8 = np.zeros((P, 2 * int(offs[-1])), NP8)
        for r in range(NT):
            m = P * r + np.arange(P)[:, None]
            nn = nidx[None, P * r:]
            diff = tsq[b][nn] - ts[b][m]
            blk = (pos_w[nn - m + (N - 1)] + ts_w[_bucket(diff)])
            blk[:, 0:P][tri] = -96.0
            w2 = np.zeros((P, 2, widths[r]), np.float32)
            w2[:, 0, :] = blk
            acc8[:, 2 * offs[r]:2 * offs[r + 1]] = \
                w2.reshape(P, 2 * widths[r]).astype(NP8)
        acc8_all.append(acc8)

    per_core = []
    for b in range(B):
        xr = ((x[b] + b_o[None, :]) * (1.0 - pad[b])[:, None]).astype(np.float16)
        per_core.append({
            "xT16": np.ascontiguousarray(x[b].T.astype(np.float16)),
            "xr": xr,
            "w_uqk8": w_uqk8, "w_uvv8": w_uvv8, "wo8": wo8, "ident8": ident8,
            "bU_col": bU_col, "bUv512": bUv512,
            "ga_col": ga_col, "bb_col": ba_col,
            "vscale_col": np.ascontiguousarray(
                ((1.0 - pad[b]) / N).astype(np.float32).reshape(NT, P).T),
            "padout_col": np.ascontiguousarray(
                (1.0 - pad[b]).astype(np.float32).reshape(NT, P).T),
            "acc8": acc8_all[b],
        })
    return per_core, (fold_ga, has_bb, haspad, has_bu)


def kernel(**inputs):
    from concourse.bass_utils import run_bass_kernel_spmd

    per_core, key = _prep_inputs(inputs)
    if key not in _cache:
        _cache.clear()
        _cache[key] = _build(*key)
    nc = _cache[key]
    res = run_bass_kernel_spmd(nc, per_core, list(range(B)))
    out = np.stack([res.results[b]["out"] for b in range(B)], axis=0)
    return out.astype(np.float32)
